# revision 1
# baseline (speedup 1.0000x reference)
"""Distributed IrisAxisVisionEncoderLayer on 8 NeuronCores.

Strategy (self-contained; shapes hardcoded from the problem spec):
  x: (B=2, C=128, X=48, Y=48, Z=48) fp32.
  x2 = 2*x + px + py + pz  (host; trivially cheap elementwise prologue)
  Stage 1  (shard Z across 8 cores, 6 planes each):
      s1 = attn_X(x2) + attn_Y(x2) + x2          (X and Y are full per shard)
  Host reshard Z-shard -> X-shard.
  Stage 2  (shard X across 8 cores):
      out = LN2(MLP(LN1(s1 + attn_Z(x2))) + LN1(s1 + attn_Z(x2)))
  Weights are replicated to every core.
"""

import numpy as np

HEADS = 8
EPS = 1e-5
NCORES = 8

_DEVICES = None
_STAGE1 = None
_STAGE2 = None


def _devices():
    global _DEVICES
    if _DEVICES is None:
        import jax
        devs = [d for d in jax.devices() if d.platform != "cpu"]
        if len(devs) < NCORES:
            devs = jax.devices()
        _DEVICES = devs[:NCORES]
    return _DEVICES


def _attn_axis(x2, axis, wq, wkv, wout, bout):
    """x2: (B, C, X, Y, Zs) jax array; attention along `axis` (2, 3, or 4)."""
    import jax
    import jax.numpy as jnp

    perm = [0] + [d for d in (2, 3, 4) if d != axis] + [axis, 1]
    xp = jnp.transpose(x2, perm)                   # (B, o1, o2, T, C)
    B, o1, o2, T, C = xp.shape
    xf = xp.reshape(B * o1 * o2, T, C)
    q = xf @ wq.T
    k, v = jnp.split(xf @ wkv.T, 2, axis=-1)
    e = C // HEADS
    sh = lambda t: t.reshape(t.shape[0], T, HEADS, e)
    q, k, v = sh(q), sh(k), sh(v)
    dots = jnp.einsum('bihe,bjhe->bhij', q, k) * (e ** -0.5)
    attn = jax.nn.softmax(dots, axis=-1)
    o = jnp.einsum('bhij,bjhe->bihe', attn, v).reshape(B * o1 * o2, T, C)
    o = (o @ wout.T + bout).reshape(B, o1, o2, T, C)
    inv = tuple(int(i) for i in np.argsort(perm))
    return jnp.transpose(o, inv)


def _stage1_fn(x2, wq0, wkv0, wout0, bout0, wq1, wkv1, wout1, bout1):
    # x2 is a Z-shard: (2, 128, 48, 48, 6). X- and Y-axis attention are local.
    a = _attn_axis(x2, 2, wq0, wkv0, wout0, bout0)
    a = a + _attn_axis(x2, 3, wq1, wkv1, wout1, bout1)
    return a + x2


def _ln(x, g, b):
    import jax
    import jax.numpy as jnp
    m = jnp.mean(x, -1, keepdims=True)
    v = jnp.mean((x - m) ** 2, -1, keepdims=True)
    return (x - m) * jax.lax.rsqrt(v + EPS) * g + b


def _stage2_fn(x2, s1, wq2, wkv2, wout2, bout2, ln_g, ln_b, w1, b1, w2, b2):
    # x2, s1 are X-shards: (2, 128, 6, 48, 48). Z-axis attention is local.
    import jax.numpy as jnp
    import jax
    x = s1 + _attn_axis(x2, 4, wq2, wkv2, wout2, bout2)
    x = jnp.transpose(x, (0, 2, 3, 4, 1))          # channels last
    x = _ln(x, ln_g, ln_b)
    res = x
    h = jax.nn.relu(x @ w1.T + b1)
    h = h @ w2.T + b2
    x = _ln(h + res, ln_g, ln_b)
    return jnp.transpose(x, (0, 4, 1, 2, 3))


def _get_jits():
    global _STAGE1, _STAGE2
    if _STAGE1 is None:
        import jax
        _STAGE1 = jax.jit(_stage1_fn)
        _STAGE2 = jax.jit(_stage2_fn)
    return _STAGE1, _STAGE2


def kernel(x, px, py, pz,
           wq0, wkv0, wout0, bout0,
           wq1, wkv1, wout1, bout1,
           wq2, wkv2, wout2, bout2,
           ln_g, ln_b, w1, b1, w2, b2):
    import jax

    devs = _devices()
    stage1, stage2 = _get_jits()

    x2 = (2.0 * np.asarray(x, np.float32)
          + np.asarray(px, np.float32)
          + np.asarray(py, np.float32)
          + np.asarray(pz, np.float32))

    w_s1 = [np.asarray(a, np.float32)
            for a in (wq0, wkv0, wout0, bout0, wq1, wkv1, wout1, bout1)]
    w_s2 = [np.asarray(a, np.float32)
            for a in (wq2, wkv2, wout2, bout2, ln_g, ln_b, w1, b1, w2, b2)]

    # ---- Stage 1: Z-shard ----
    zc = x2.shape[4] // NCORES
    futs = []
    for c in range(NCORES):
        shard = x2[:, :, :, :, c * zc:(c + 1) * zc]
        args = [jax.device_put(shard, devs[c])] + [
            jax.device_put(w, devs[c]) for w in w_s1]
        futs.append(stage1(*args))
    s1 = np.concatenate([np.asarray(f) for f in futs], axis=4)

    # ---- Stage 2: X-shard ----
    xc = x2.shape[2] // NCORES
    futs = []
    for c in range(NCORES):
        x2s = x2[:, :, c * xc:(c + 1) * xc, :, :]
        s1s = s1[:, :, c * xc:(c + 1) * xc, :, :]
        args = [jax.device_put(x2s, devs[c]), jax.device_put(s1s, devs[c])] + [
            jax.device_put(w, devs[c]) for w in w_s2]
        futs.append(stage2(*args))
    out = np.concatenate([np.asarray(f) for f in futs], axis=2)
    return out.astype(np.float32)
